# revision 8
# baseline (speedup 1.0000x reference)
"""TransformerXL attention (AttentionXL) Bass kernel for Trainium2, 8 NeuronCores.

Sharding: pure data-parallel over batch (BS=8 -> 1 batch element per core).
All weights replicated per core; no collectives.

v2 design notes (vs v1 baseline at ~389us):
  - A^T transposes moved off the TensorEngine onto the DMA XBAR
    (dma_start(transpose=True)): one instruction per (head, i-block)
    transposes [128, jmax] -> [j%128, jc, i] directly into SBUF.
  - C+S done in-place in PSUM (DVE add), exp reads PSUM directly.
  - Causal mask applied to the S tile (gpsimd affine_select) on the single
    diagonal 128-col block per i-block, *before* the add - off the critical
    path, and 4x less gpsimd work than masking [512:jmax].
  - P matmuls trimmed to the m-range actually read by the rel-shift
    (m >= 384-128*ib), except heads 0/1 which write the full range so the
    p_all pool buffers never expose uninitialized SBUF to the DMA.
  - P scores pipelined TWO heads ahead of the C/softmax pipeline so the
    P->DRAM->S (rel-shift) DMA roundtrip never stalls the PE.
  - Initial weight/activation DMAs split per 128-row chunk; first QT
    matmuls start after ~2 chunks instead of after the full 13.6MB load.
  - PSUM->SBUF copy work balanced across scalar/gpsimd/DVE.

Per-core algorithm (bf16 on the PE, fp32 PSUM accumulation):
  Host prep:  X^T, Xc^T, Pos^T, W_kv split into W_k/W_v, bias folds:
                bias_qu = b_q + u.ravel(), bias_qv = b_q + v.ravel()
                b_out   = b_v @ W_proj + b_proj  (softmax rows sum to 1)
  Device:
    KT = W_k^T @ X^T   [hd, j]   (+b_k)      RT = W_pos^T @ P^T [hd, m]
    QT = W_q^T @ Xc^T  [hd, i]  -> QuT/QvT   V  = X^T.T @ W_v   [j, hd]
    per head h:
      P  [i,m] = QvT_h^T RT_h  -> DRAM;  S [i,j] = Pflat[i*1023 + 511 + j]
      mask S diagonal blocks; C [i,j] = QuT_h^T KT_h (PSUM); C += S (DVE);
      A = exp(C*0.125) accum Z (ScalarE from PSUM); A *= 1/Z (DVE)
      A^T via DMA XBAR transpose -> a_t [j, i]
      O^T_h [d, i] = V_h^T A^T (PE, col-packed head pairs) -> AVT
    out[i,e] = AVT^T @ W_proj + b_out (ones-row bias matmul), fp32.
"""

import os
import sys

for _p in (
    "/root/.axon_site",
    "/root/.axon_site/_ro/trn_rl_repo",
    "/root/.axon_site/_ro/pypackages",
    "/opt/trn_rl_repo",
):
    if os.path.isdir(_p) and _p not in sys.path:
        sys.path.append(_p)

import numpy as np
import ml_dtypes

import concourse.bass as bass
import concourse.mybir as mybir
import concourse.tile as tile
from concourse.bass_utils import run_bass_kernel_spmd

BF16 = mybir.dt.bfloat16
FP32 = mybir.dt.float32
AF = mybir.ActivationFunctionType
ALU = mybir.AluOpType
nbf16 = ml_dtypes.bfloat16

CUR, FULL, BS, DIM, H, D = 512, 1024, 8, 1024, 16, 64
PREV = FULL - CUR
SCALE = 1.0 / D**0.5
P = 128
NIB = CUR // P    # 4 query blocks
NJC = FULL // P   # 8 key chunks
NCH = DIM // P    # 8 dim chunks
NHP = H // 2      # 8 head pairs
MASK_FILL = -30000.0

_BUILT = None


def _split_multiwait(nc):
    """walrus here encodes at most ONE sync wait per TPB instruction
    (NEURON_ISA_TPB_EVENTS has a single wait slot).  Split every
    multi-wait instruction: prepend same-engine NoOps carrying the
    extra waits, keep the last wait on the instruction itself."""
    n_split = 0
    for fn in nc.m.functions:
        for blk in fn.blocks:
            insts = list(blk.instructions)
            out = []
            for ins in insts:
                si = ins.sync_info
                if si is not None and si.on_wait and len(si.on_wait) > 1:
                    waits = list(si.on_wait)
                    for w in waits[:-1]:
                        nop = mybir.InstNoOp(
                            name=f"{ins.name}-ws{n_split}",
                            engine=ins.engine,
                            sync_info=mybir.SyncInfo(on_wait=[w], on_update=[]),
                            text_hint="waitsplit",
                        )
                        out.append(nop)
                        n_split += 1
                    ins.sync_info = mybir.SyncInfo(
                        on_wait=[waits[-1]],
                        on_update=list(si.on_update or []),
                    )
                out.append(ins)
            blk.instructions = out
    return n_split


def _build(split_waits=True):
    nc = bass.Bass()

    # acts: [X^T | Xc^T | Pos^T] cols; wmats: [W_q | W_pos | W_k | W_v] cols
    acts = nc.declare_dram_parameter("acts", [DIM, FULL + CUR + FULL], BF16, isOutput=False)
    wmats = nc.declare_dram_parameter("wmats", [DIM, 4 * DIM], BF16, isOutput=False)
    wproj = nc.declare_dram_parameter("wproj", [DIM, DIM], BF16, isOutput=False)
    # biases pre-laid-out on host: [p, 4*NCH] = qu | qv | k | pos chunks
    biases = nc.declare_dram_parameter("biases", [P, 4 * NCH], FP32, isOutput=False)
    bout = nc.declare_dram_parameter("bout", [DIM], BF16, isOutput=False)
    out = nc.declare_dram_parameter("out", [CUR, DIM], FP32, isOutput=True)

    ACTS_W = FULL + CUR + FULL

    with tile.TileContext(nc) as tc:
        from contextlib import ExitStack

        with ExitStack() as ctx:
            persist = ctx.enter_context(tc.tile_pool(name="persist", bufs=1))

            KT = persist.tile([P, NCH, FULL], BF16, tag="KT")
            RT = persist.tile([P, NCH, FULL], BF16, tag="RT")
            V = persist.tile([P, NJC, DIM], BF16, tag="V")
            QuT = persist.tile([P, NCH, CUR], BF16, tag="QuT")
            QvT = persist.tile([P, NCH, CUR], BF16, tag="QvT")
            AVT = persist.tile([P, NCH, CUR], BF16, tag="AVT")
            ones_row = persist.tile([P, P], BF16, tag="ones_row")
            bout_t = persist.tile([P, DIM], BF16, tag="bout_t")
            bias_t = persist.tile([P, 4, NCH], FP32, tag="bias_t")  # qu|qv|k|pos

            mask_fill_reg = nc.gpsimd.to_reg(MASK_FILL)
            nc.vector.memset(ones_row, 0.0)
            nc.vector.memset(ones_row[0:1, :], 1.0)
            nc.vector.memset(bout_t, 0.0)
            nc.sync.dma_start(bias_t, biases.rearrange("p (b c) -> p b c", b=4))
            nc.sync.dma_start(bout_t[0:1, :], bout[None, :])

            # ---------------- Stage A: projections ----------------
            with tc.tile_pool(name="ain", bufs=1) as ain, tc.tile_pool(
                name="apsum", bufs=4, space="PSUM"
            ) as apsum:
                acts_t = ain.tile([P, NCH, ACTS_W], BF16, tag="acts")
                wmats_t = ain.tile([P, NCH, 4 * DIM], BF16, tag="wmats")
                # split input loads per chunk so the PE can start early
                acts_r = acts.rearrange("(c p) f -> p c f", p=P)
                wmats_r = wmats.rearrange("(c p) f -> p c f", p=P)
                for c in range(NCH):
                    nc.sync.dma_start(wmats_t[:, c : c + 1], wmats_r[:, c : c + 1])
                    nc.sync.dma_start(acts_t[:, c : c + 1], acts_r[:, c : c + 1])

                xT_t = acts_t[:, :, 0:FULL]
                xcT_t = acts_t[:, :, FULL : FULL + CUR]
                pT_t = acts_t[:, :, FULL + CUR : FULL + CUR + FULL]
                wq_t = wmats_t[:, :, 0:DIM]
                wpos_t = wmats_t[:, :, DIM : 2 * DIM]
                wk_t = wmats_t[:, :, 2 * DIM : 3 * DIM]
                wv_t = wmats_t[:, :, 3 * DIM : 4 * DIM]

                # Q^T [hd, i]: two groups of 4 oc tiles, kc-outer inside the
                # group, so early matmuls consume input chunks as they land.
                for grp in range(2):
                    ocs = range(grp * 4, grp * 4 + 4)
                    pss = {
                        oc: apsum.tile([P, CUR], FP32, tag="aps", name=f"qps{oc}")
                        for oc in ocs
                    }
                    for kc in range(NCH):
                        for oc in ocs:
                            nc.tensor.matmul(
                                pss[oc],
                                wq_t[:, kc, oc * P : (oc + 1) * P],
                                xcT_t[:, kc, :],
                                start=(kc == 0),
                                stop=(kc == NCH - 1),
                            )
                    for oc in ocs:
                        nc.scalar.activation(
                            QuT[:, oc, :], pss[oc], AF.Identity,
                            bias=bias_t[:, 0, oc : oc + 1],
                        )
                        nc.scalar.activation(
                            QvT[:, oc, :], pss[oc], AF.Identity,
                            bias=bias_t[:, 1, oc : oc + 1],
                        )

                # K^T [hd, j] and R^T [hd, m]
                for oc in range(NCH):
                    for jh in range(2):
                        sl = slice(jh * 512, (jh + 1) * 512)
                        ps = apsum.tile([P, 512], FP32, tag="aps")
                        for kc in range(NCH):
                            nc.tensor.matmul(
                                ps,
                                wk_t[:, kc, oc * P : (oc + 1) * P],
                                xT_t[:, kc, sl],
                                start=(kc == 0),
                                stop=(kc == NCH - 1),
                            )
                        nc.scalar.activation(
                            KT[:, oc, sl], ps, AF.Identity,
                            bias=bias_t[:, 2, oc : oc + 1],
                        )
                        ps = apsum.tile([P, 512], FP32, tag="aps")
                        for kc in range(NCH):
                            nc.tensor.matmul(
                                ps,
                                wpos_t[:, kc, oc * P : (oc + 1) * P],
                                pT_t[:, kc, sl],
                                start=(kc == 0),
                                stop=(kc == NCH - 1),
                            )
                        nc.scalar.activation(
                            RT[:, oc, sl], ps, AF.Identity,
                            bias=bias_t[:, 3, oc : oc + 1],
                        )

                # V [j, hd]
                for jc in range(NJC):
                    for mh in range(2):
                        sl = slice(mh * 512, (mh + 1) * 512)
                        ps = apsum.tile([P, 512], FP32, tag="aps")
                        for kc in range(NCH):
                            nc.tensor.matmul(
                                ps,
                                xT_t[:, kc, jc * P : (jc + 1) * P],
                                wv_t[:, kc, sl],
                                start=(kc == 0),
                                stop=(kc == NCH - 1),
                            )
                        nc.vector.tensor_copy(V[:, jc, sl], ps)

            # ---------------- Stage B: attention per head ----------------
            late = ctx.enter_context(tc.tile_pool(name="late", bufs=1))
            pall = ctx.enter_context(tc.tile_pool(name="pall", bufs=2))
            sall = ctx.enter_context(tc.tile_pool(name="sall", bufs=3))
            asb = ctx.enter_context(tc.tile_pool(name="asb", bufs=4))
            atp = ctx.enter_context(tc.tile_pool(name="atp", bufs=2))
            work = ctx.enter_context(tc.tile_pool(name="work", bufs=4))
            dram = ctx.enter_context(tc.tile_pool(name="dram", bufs=3, space="DRAM"))
            pps = ctx.enter_context(tc.tile_pool(name="pps", bufs=2, space="PSUM"))
            cps = ctx.enter_context(tc.tile_pool(name="cps", bufs=2, space="PSUM"))
            avp = ctx.enter_context(tc.tile_pool(name="avp", bufs=1, space="PSUM"))

            WPROJ = late.tile([P, NCH, DIM], BF16, tag="WPROJ")
            nc.sync.dma_start(WPROJ, wproj.rearrange("(c p) f -> p c f", p=P))

            s_tiles = [None] * (H + 2)   # s_all tile per head
            at_tiles = [None] * H        # a_t tile per head
            pall_cur = [None]            # p_all tile for the head being P-emitted

            def emit_P_ib(h, ib):
                """P-score matmuls + PSUM->SBUF copies for (head h, i-block ib)."""
                ch, ro = divmod(h, 2)
                ro *= D
                rs = slice(ro, ro + D)
                isl = slice(ib * P, (ib + 1) * P)
                if ib == 0:
                    pall_cur[0] = pall.tile([P, NIB, FULL], BF16, tag="p_all", name="p_all")
                p_all = pall_cur[0]
                # full m-range: the write DMA reads the whole tile and the
                # race detector requires every byte to come from this head's
                # writes (the rel-shift only consumes m >= 384-128*ib, but
                # trimming would leave stale bytes under the DMA read).
                mlo = 0
                for mh in range(2):
                    m0, m1 = max(mh * 512, mlo), (mh + 1) * 512
                    if m1 <= m0:
                        continue
                    w = m1 - m0
                    pp = pps.tile([P, 512], FP32, tag="pp")
                    nc.tensor.matmul(
                        pp[:, :w], QvT[rs, ch, isl], RT[rs, ch, m0:m1],
                        start=True, stop=True,
                    )
                    # gpsimd cannot read PSUM (BIR verifier); split scalar/DVE
                    if ib < 2:
                        nc.scalar.copy(p_all[:, ib, m0:m1], pp[:, :w])
                    else:
                        nc.vector.tensor_copy(p_all[:, ib, m0:m1], pp[:, :w])

            def emit_P_post(h):
                """DRAM roundtrip (rel-shift) + diagonal mask for head h."""
                p_all = pall_cur[0]
                pdram = dram.tile([CUR, FULL], BF16, tag="pdram")
                nc.sync.dma_start(
                    pdram.rearrange("(ib p) m -> p ib m", p=P), p_all
                )
                # shifted read: S[ib*128+u, j] = Pflat[(ib*128+u)*1023 + 511 + j]
                s_all = sall.tile([P, NIB, FULL], BF16, tag="s_all", name="s_all")
                sh_ap = bass.AP(
                    tensor=pdram.tensor,
                    offset=pdram.offset + (PREV - 1),
                    ap=[[FULL - 1, P], [(FULL - 1) * P, NIB], [1, FULL]],
                )
                nc.sync.dma_start(s_all, sh_ap)
                # causal mask: only the diagonal 128-col block per i-block is
                # mixed; j < 512+128*ib is fully valid, j >= 640+128*ib unread.
                # keep iff u - j' >= 0 (u = partition = i%128, j' = j-512-128*ib)
                for ib in range(NIB):
                    j0 = 512 + ib * P
                    nc.gpsimd.affine_select(
                        out=s_all[:, ib, j0 : j0 + P],
                        in_=s_all[:, ib, j0 : j0 + P],
                        compare_op=ALU.is_ge,
                        fill=mask_fill_reg,
                        base=0,
                        channel_multiplier=1,
                        pattern=[[-1, P]],
                    )
                s_tiles[h] = s_all

            def emit_C_ib(h, ib):
                """C matmuls, +S, mask-free softmax, XBAR transpose."""
                ch, ro = divmod(h, 2)
                ro *= D
                rs = slice(ro, ro + D)
                isl = slice(ib * P, (ib + 1) * P)
                jmax = 640 + ib * P
                s_all = s_tiles[h]
                if ib == 0:
                    at_tiles[h] = atp.tile([P, NJC, CUR], BF16, tag=f"at{h % 2}", name=f"at{h % 2}")
                a_t = at_tiles[h]
                cp = cps.tile([P, FULL], FP32, tag="cp")
                nc.tensor.matmul(
                    cp[:, 0:512], QuT[rs, ch, isl], KT[rs, ch, 0:512],
                    start=True, stop=True,
                )
                nc.tensor.matmul(
                    cp[:, 512:jmax], QuT[rs, ch, isl], KT[rs, ch, 512:jmax],
                    start=True, stop=True,
                )
                nc.vector.tensor_tensor(
                    cp[:, :jmax], cp[:, :jmax], s_all[:, ib, :jmax], ALU.add
                )
                a_sb = asb.tile([P, FULL], BF16, tag="a_sb")
                z_t = work.tile([P, 1], FP32, tag="z_t")
                nc.scalar.activation(
                    a_sb[:, :jmax], cp[:, :jmax], AF.Exp,
                    scale=SCALE, accum_out=z_t,
                )
                rz = work.tile([P, 1], FP32, tag="rz")
                nc.vector.reciprocal(rz, z_t)
                # normalize on gpsimd (SBUF->SBUF, keeps DVE/scalar headroom)
                nc.gpsimd.tensor_scalar_mul(a_sb[:, :jmax], a_sb[:, :jmax], rz)
                # full-width transpose on the DMA XBAR: [128 i, jmax] ->
                # [j%128, jc, i] straight into the a_t tile.
                nc.sync.dma_start(
                    a_t[:, 0 : jmax // P, isl], a_sb[:, :jmax], transpose=True
                )

            def emit_AV(hp):
                """O^T for the head pair: col-packed accumulation over jc."""
                av2 = [avp.tile([P, CUR], FP32, tag="av_a", name="av_a"),
                       avp.tile([P, CUR], FP32, tag="av_b", name="av_b")]
                for jc in range(NJC):
                    ilo = max(0, (jc - 4)) * P
                    for hh in range(2):
                        h = 2 * hp + hh
                        nc.tensor.matmul(
                            av2[hh][hh * D : (hh + 1) * D, ilo:],
                            V[:, jc, h * D : (h + 1) * D],
                            at_tiles[h][:, jc, ilo:],
                            start=(jc == 0),
                            stop=(jc == NJC - 1),
                            tile_position=(0, hh * D),
                        )
                nc.vector.tensor_copy(AVT[0:D, hp, :], av2[0][0:D, :])
                nc.vector.tensor_copy(AVT[D:P, hp, :], av2[1][D:P, :])

            # prologue: P for heads 0 and 1
            for hh in range(2):
                for ib in range(NIB):
                    emit_P_ib(hh, ib)
                emit_P_post(hh)

            for h in range(H):
                for ib in range(NIB):
                    if h + 2 < H:
                        emit_P_ib(h + 2, ib)
                    emit_C_ib(h, ib)
                if h + 2 < H:
                    emit_P_post(h + 2)
                s_tiles[h] = None
                if h % 2 == 1:
                    emit_AV(h // 2)

            # ---------------- Final projection ----------------
            with tc.tile_pool(name="fin", bufs=1) as fin:
                o_all = fin.tile([P, NIB, DIM], FP32, tag="o_all")
                for ib in range(NIB):
                    isl = slice(ib * P, (ib + 1) * P)
                    for eh in range(2):
                        esl = slice(eh * 512, (eh + 1) * 512)
                        fp = avp.tile([P, 512], FP32, tag=("av_a", "av_b")[eh], name="fp")
                        for fc in range(NCH):
                            nc.tensor.matmul(
                                fp, AVT[:, fc, isl], WPROJ[:, fc, esl],
                                start=(fc == 0), stop=False,
                            )
                        nc.tensor.matmul(
                            fp, ones_row, bout_t[:, esl], start=False, stop=True
                        )
                        nc.vector.tensor_copy(o_all[:, ib, esl], fp)
                nc.sync.dma_start(out.rearrange("(ib p) e -> p ib e", p=P), o_all)

    if split_waits:
        _split_multiwait(nc)
    return nc


def _get_nc():
    global _BUILT
    if _BUILT is None:
        _BUILT = _build()
    return _BUILT


def _prep_host(inputs, pos_embedding, full_input, u, v, mask,
               W_kv, b_kv, W_q, b_q, W_pos, b_pos, W_proj, b_proj):
    f32 = np.float32
    W_k = np.ascontiguousarray(W_kv[:, : H * D])
    W_v = np.ascontiguousarray(W_kv[:, H * D :])
    b_k = b_kv[: H * D].astype(f32)
    b_v = b_kv[H * D :].astype(f32)
    bias_qu = (b_q + u.ravel()).astype(f32)
    bias_qv = (b_q + v.ravel()).astype(f32)
    b_out = (b_v @ W_proj + b_proj).astype(f32)

    bias_all = np.stack(
        [bias_qu.reshape(NCH, P), bias_qv.reshape(NCH, P),
         b_k.reshape(NCH, P), b_pos.astype(f32).reshape(NCH, P)], axis=0
    )  # [4, NCH, P]
    bias_all = np.ascontiguousarray(bias_all.transpose(2, 0, 1).reshape(P, 4 * NCH))
    wmats_np = np.concatenate([W_q, W_pos, W_k, W_v], axis=1).astype(nbf16)
    shared = {
        "wmats": wmats_np,
        "wproj": W_proj.astype(nbf16),
        "biases": bias_all.astype(f32),
        "bout": b_out.astype(nbf16),
    }
    pT_np = pos_embedding[:, 0].T
    in_maps = []
    for c in range(BS):
        m = dict(shared)
        m["acts"] = np.concatenate(
            [full_input[:, c].T, inputs[:, c].T, pT_np], axis=1
        ).astype(nbf16)
        in_maps.append(m)
    return in_maps


def kernel(**inputs):
    nc = _get_nc()
    in_maps = _prep_host(**{k: np.asarray(v) for k, v in inputs.items()})
    res = run_bass_kernel_spmd(nc, in_maps, list(range(BS)))
    out = np.stack([res.results[c]["out"] for c in range(BS)], axis=1)
    return np.ascontiguousarray(out.astype(np.float32))


if __name__ == "__main__":
    nc = _build()
    print("built ok")


# revision 11
# speedup vs baseline: 2.7223x; 2.7223x over previous
"""TransformerXL attention (AttentionXL) Bass kernel for Trainium2, 8 NeuronCores.

Sharding: pure data-parallel over batch (BS=8 -> 1 batch element per core).
All weights replicated per core; no collectives.

v5 design (evolution of the ~389us baseline):
  - exp factoring: A = exp((C+S)*s) = exp(C*s) * exp(S*s).  The position
    scores are exponentiated ON THE WAY OUT of PSUM (the copy becomes the
    exp), the rel-shift DMA gathers exp(S*s), and the C-side combine is a
    bf16*bf16 SBUF multiply on the DVE (2x rate) with the softmax
    denominator accumulated in the same instruction (tensor_tensor_reduce).
    The causal mask becomes a multiplicative 0-fill on the diagonal block.
  - P-score production (matmuls + expP + DRAM write) is woven INTO stage A
    right after each RT chunk is ready, using otherwise-idle scalar-engine
    time; the gather stays two heads ahead of the C pipeline.
  - A^T via ONE DMA XBAR transpose per head ([128 i, 4096 (ib,j)] ->
    [j%128, (ib,jc), i%128]); the AV matmuls read [part, ib-dim, u-dim]
    3-level APs.  No PE transposes at all.
  - P matmuls/exps trimmed to the m-range the rel-shift reads
    (m >= 384-128*ib); persistent p_all/a_sb tiles with one-time memsets
    keep every byte under the DMAs initialized (race-detector clean).
  - Final projection bias via a broadcast b_out tile + DVE add (the
    PSUM->SBUF copy becomes the add); no ones-row bias matmul.
  - Initial weight/activation DMAs split per 128-row chunk; QT computed
    kc-outer in two 4-tile groups so the PE starts as chunks land.

Per-core algorithm (bf16 on the PE, fp32 PSUM accumulation):
  Host prep:  X^T, Xc^T, Pos^T, W_kv split into W_k/W_v, bias folds:
                bias_qu = b_q + u.ravel(), bias_qv = b_q + v.ravel()
                b_out   = b_v @ W_proj + b_proj  (softmax rows sum to 1)
  Device:
    KT = W_k^T @ X^T   [hd, j]   (+b_k)      RT = W_pos^T @ P^T [hd, m]
    QT = W_q^T @ Xc^T  [hd, i]  -> QuT/QvT   V  = X^T.T @ W_v   [j, hd]
    per head h:
      eP [i,m] = exp(QvT_h^T RT_h * s) -> DRAM
      eS [i,j] = ePflat[i*1023 + 511 + j]  (rel-shift gather), diag 0-mask
      eC [i,j] = exp(QuT_h^T KT_h * s)  (ScalarE from PSUM)
      A = eC*eS, Z = sum_j A  (DVE tensor_tensor_reduce);  A *= 1/Z
      A^T via DMA XBAR transpose -> a_t [j%128, (ib,jc), i%128]
      O^T_h [d, i] = V_h^T A^T (PE, col-packed head pairs) -> AVT
    out[i,e] = AVT^T @ W_proj (+ b_out via DVE broadcast add), fp32.
"""

import os
import sys

for _p in (
    "/root/.axon_site",
    "/root/.axon_site/_ro/trn_rl_repo",
    "/root/.axon_site/_ro/pypackages",
    "/opt/trn_rl_repo",
):
    if os.path.isdir(_p) and _p not in sys.path:
        sys.path.append(_p)

import numpy as np
import ml_dtypes

import concourse.bass as bass
import concourse.mybir as mybir
import concourse.tile as tile
from concourse.bass_utils import run_bass_kernel_spmd

BF16 = mybir.dt.bfloat16
FP32 = mybir.dt.float32
AF = mybir.ActivationFunctionType
ALU = mybir.AluOpType
nbf16 = ml_dtypes.bfloat16

CUR, FULL, BS, DIM, H, D = 512, 1024, 8, 1024, 16, 64
PREV = FULL - CUR
SCALE = 1.0 / D**0.5
P = 128
NIB = CUR // P    # 4 query blocks
NJC = FULL // P   # 8 key chunks
NCH = DIM // P    # 8 dim chunks
NHP = H // 2      # 8 head pairs

_BUILT = None


def _mlo(ib):
    # lowest m the rel-shift gather reads within i-block ib
    return max(0, 384 - 128 * ib)


def _split_multiwait(nc):
    """walrus here encodes at most ONE sync wait per TPB instruction
    (NEURON_ISA_TPB_EVENTS has a single wait slot).  Split every
    multi-wait instruction: prepend same-engine NoOps carrying the
    extra waits, keep the last wait on the instruction itself."""
    n_split = 0
    for fn in nc.m.functions:
        for blk in fn.blocks:
            insts = list(blk.instructions)
            out = []
            for ins in insts:
                si = ins.sync_info
                if si is not None and si.on_wait and len(si.on_wait) > 1:
                    waits = list(si.on_wait)
                    for w in waits[:-1]:
                        nop = mybir.InstNoOp(
                            name=f"{ins.name}-ws{n_split}",
                            engine=ins.engine,
                            sync_info=mybir.SyncInfo(on_wait=[w], on_update=[]),
                            text_hint="waitsplit",
                        )
                        out.append(nop)
                        n_split += 1
                    ins.sync_info = mybir.SyncInfo(
                        on_wait=[waits[-1]],
                        on_update=list(si.on_update or []),
                    )
                out.append(ins)
            blk.instructions = out
    return n_split


def _build(split_waits=True):
    nc = bass.Bass()

    # acts: [X^T | Xc^T | Pos^T] cols; wmats: [W_q | W_pos | W_k | W_v] cols
    acts = nc.declare_dram_parameter("acts", [DIM, FULL + CUR + FULL], BF16, isOutput=False)
    wmats = nc.declare_dram_parameter("wmats", [DIM, 4 * DIM], BF16, isOutput=False)
    wproj = nc.declare_dram_parameter("wproj", [DIM, DIM], BF16, isOutput=False)
    # biases pre-laid-out on host: [p, 4*NCH] = qu | qv | k | pos chunks
    biases = nc.declare_dram_parameter("biases", [P, 4 * NCH], FP32, isOutput=False)
    bout = nc.declare_dram_parameter("bout", [DIM], BF16, isOutput=False)
    out = nc.declare_dram_parameter("out", [CUR, DIM], FP32, isOutput=True)

    ACTS_W = FULL + CUR + FULL

    with tile.TileContext(nc) as tc:
        from contextlib import ExitStack

        with ExitStack() as ctx:
            persist = ctx.enter_context(tc.tile_pool(name="persist", bufs=1))

            KT = persist.tile([P, NCH, FULL], BF16, tag="KT")
            RT = persist.tile([P, NCH, FULL], BF16, tag="RT")
            V = persist.tile([P, NJC, DIM], BF16, tag="V")
            QuT = persist.tile([P, NCH, CUR], BF16, tag="QuT")
            QvT = persist.tile([P, NCH, CUR], BF16, tag="QvT")
            AVT = persist.tile([P, NCH, CUR], BF16, tag="AVT")
            bout_b = persist.tile([P, DIM], BF16, tag="bout_b")
            bias_t = persist.tile([P, 4, NCH], FP32, tag="bias_t")  # qu|qv|k|pos
            # persistent exp(P) staging pair (h%2); trimmed regions memset once
            pa = [persist.tile([P, NIB, FULL], BF16, tag="pa0", name="pa0"),
                  persist.tile([P, NIB, FULL], BF16, tag="pa1", name="pa1")]

            mask_zero_reg = nc.gpsimd.to_reg(0.0)
            nc.sync.dma_start(bias_t, biases.rearrange("p (b c) -> p b c", b=4))
            # broadcast b_out across all partitions (stride-0 partition read)
            nc.sync.dma_start(
                bout_b, bass.AP(tensor=bout, offset=0, ap=[[0, P], [1, DIM]])
            )
            for t in pa:
                for ib in range(NIB - 1):
                    nc.vector.memset(t[:, ib, 0 : _mlo(ib)], 0.0)

            pdram_tiles = [None] * H
            sexp_tiles = [None] * H
            # DRAM scratch outlives stage A (gathers happen in stage B)
            dram = ctx.enter_context(tc.tile_pool(name="dram", bufs=16, space="DRAM"))

            # ---------------- Stage A + P-score production ----------------
            with tc.tile_pool(name="ain", bufs=1) as ain, tc.tile_pool(
                name="apsum", bufs=4, space="PSUM"
            ) as apsum, tc.tile_pool(name="pps", bufs=2, space="PSUM") as pps:
                acts_t = ain.tile([P, NCH, ACTS_W], BF16, tag="acts")
                wmats_t = ain.tile([P, NCH, 4 * DIM], BF16, tag="wmats")
                acts_r = acts.rearrange("(c p) f -> p c f", p=P)
                wmats_r = wmats.rearrange("(c p) f -> p c f", p=P)
                for c in range(NCH):
                    nc.sync.dma_start(wmats_t[:, c : c + 1], wmats_r[:, c : c + 1])
                    nc.sync.dma_start(acts_t[:, c : c + 1], acts_r[:, c : c + 1])

                xT_t = acts_t[:, :, 0:FULL]
                xcT_t = acts_t[:, :, FULL : FULL + CUR]
                pT_t = acts_t[:, :, FULL + CUR : FULL + CUR + FULL]
                wq_t = wmats_t[:, :, 0:DIM]
                wpos_t = wmats_t[:, :, DIM : 2 * DIM]
                wk_t = wmats_t[:, :, 2 * DIM : 3 * DIM]
                wv_t = wmats_t[:, :, 3 * DIM : 4 * DIM]

                # Q^T [hd, i]: two groups of 4 oc tiles, kc-outer inside the
                # group, so early matmuls consume input chunks as they land.
                for grp in range(2):
                    ocs = range(grp * 4, grp * 4 + 4)
                    pss = {
                        oc: apsum.tile([P, CUR], FP32, tag="aps", name=f"qps{oc}")
                        for oc in ocs
                    }
                    for kc in range(NCH):
                        for oc in ocs:
                            nc.tensor.matmul(
                                pss[oc],
                                wq_t[:, kc, oc * P : (oc + 1) * P],
                                xcT_t[:, kc, :],
                                start=(kc == 0),
                                stop=(kc == NCH - 1),
                            )
                    for oc in ocs:
                        nc.scalar.activation(
                            QuT[:, oc, :], pss[oc], AF.Identity,
                            bias=bias_t[:, 0, oc : oc + 1],
                        )
                        nc.scalar.activation(
                            QvT[:, oc, :], pss[oc], AF.Identity,
                            bias=bias_t[:, 1, oc : oc + 1],
                        )

                def emit_P(h):
                    """exp(P*s) production for head h: matmuls + expP + DRAM."""
                    ch, ro = divmod(h, 2)
                    ro *= D
                    rs = slice(ro, ro + D)
                    p_all = pa[h % 2]
                    for ib in range(NIB):
                        isl = slice(ib * P, (ib + 1) * P)
                        mlo = _mlo(ib)
                        pp = pps.tile([P, FULL], FP32, tag="pp", name="pp")
                        nc.tensor.matmul(
                            pp[:, mlo:512], QvT[rs, ch, isl], RT[rs, ch, mlo:512],
                            start=True, stop=True,
                        )
                        nc.tensor.matmul(
                            pp[:, 512:FULL], QvT[rs, ch, isl], RT[rs, ch, 512:FULL],
                            start=True, stop=True,
                        )
                        nc.scalar.activation(
                            p_all[:, ib, mlo:], pp[:, mlo:], AF.Exp, scale=SCALE
                        )
                    pdram = dram.tile([CUR, FULL], BF16, tag="pdram", name="pdram")
                    nc.sync.dma_start(
                        pdram.rearrange("(ib p) m -> p ib m", p=P), p_all
                    )
                    pdram_tiles[h] = pdram

                # R^T chunks, each followed by the P production it unblocks
                for ch in range(NCH):
                    for jh in range(2):
                        sl = slice(jh * 512, (jh + 1) * 512)
                        ps = apsum.tile([P, 512], FP32, tag="aps")
                        for kc in range(NCH):
                            nc.tensor.matmul(
                                ps,
                                wpos_t[:, kc, ch * P : (ch + 1) * P],
                                pT_t[:, kc, sl],
                                start=(kc == 0),
                                stop=(kc == NCH - 1),
                            )
                        nc.scalar.activation(
                            RT[:, ch, sl], ps, AF.Identity,
                            bias=bias_t[:, 3, ch : ch + 1],
                        )
                    emit_P(2 * ch)
                    emit_P(2 * ch + 1)

                # K^T [hd, j]
                for oc in range(NCH):
                    for jh in range(2):
                        sl = slice(jh * 512, (jh + 1) * 512)
                        ps = apsum.tile([P, 512], FP32, tag="aps")
                        for kc in range(NCH):
                            nc.tensor.matmul(
                                ps,
                                wk_t[:, kc, oc * P : (oc + 1) * P],
                                xT_t[:, kc, sl],
                                start=(kc == 0),
                                stop=(kc == NCH - 1),
                            )
                        nc.scalar.activation(
                            KT[:, oc, sl], ps, AF.Identity,
                            bias=bias_t[:, 2, oc : oc + 1],
                        )

                # V [j, hd]: low head-halves first (AV consumes low heads first)
                for mh in range(2):
                    sl = slice(mh * 512, (mh + 1) * 512)
                    for jc in range(NJC):
                        ps = apsum.tile([P, 512], FP32, tag="aps")
                        for kc in range(NCH):
                            nc.tensor.matmul(
                                ps,
                                xT_t[:, kc, jc * P : (jc + 1) * P],
                                wv_t[:, kc, sl],
                                start=(kc == 0),
                                stop=(kc == NCH - 1),
                            )
                        nc.vector.tensor_copy(V[:, jc, sl], ps)

            # ---------------- Stage B: attention per head ----------------
            late = ctx.enter_context(tc.tile_pool(name="late", bufs=1))
            sall = ctx.enter_context(tc.tile_pool(name="sall", bufs=3))
            atp = ctx.enter_context(tc.tile_pool(name="atp", bufs=1))
            work = ctx.enter_context(tc.tile_pool(name="work", bufs=4))
            cps = ctx.enter_context(tc.tile_pool(name="cps", bufs=3, space="PSUM"))
            avp = ctx.enter_context(tc.tile_pool(name="avp", bufs=1, space="PSUM"))

            WPROJ = late.tile([P, NCH, DIM], BF16, tag="WPROJ")
            nc.sync.dma_start(WPROJ, wproj.rearrange("(c p) f -> p c f", p=P))
            # persistent exp(C)/A staging pair; beyond-jmax regions memset once
            asb = [late.tile([P, NIB, FULL], BF16, tag="as0", name="as0"),
                   late.tile([P, NIB, FULL], BF16, tag="as1", name="as1")]
            for t in asb:
                for ib in range(NIB - 1):
                    nc.vector.memset(t[:, ib, 640 + ib * P :], 0.0)

            at_tiles = [None] * H

            def emit_gather(h):
                """rel-shift gather of exp(S*s) + multiplicative diag mask."""
                pdram = pdram_tiles[h]
                s_exp = sall.tile([P, NIB, FULL], BF16, tag="s_exp", name="s_exp")
                sh_ap = bass.AP(
                    tensor=pdram.tensor,
                    offset=pdram.offset + (PREV - 1),
                    ap=[[FULL - 1, P], [(FULL - 1) * P, NIB], [1, FULL]],
                )
                nc.sync.dma_start(s_exp, sh_ap)
                # causal mask: zero the over-diagonal in the diagonal block
                # (keep iff u - j' >= 0; u = i%128, j' = j-512-128*ib)
                for ib in range(NIB):
                    j0 = 512 + ib * P
                    nc.gpsimd.affine_select(
                        out=s_exp[:, ib, j0 : j0 + P],
                        in_=s_exp[:, ib, j0 : j0 + P],
                        compare_op=ALU.is_ge,
                        fill=mask_zero_reg,
                        base=0,
                        channel_multiplier=1,
                        pattern=[[-1, P]],
                    )
                sexp_tiles[h] = s_exp
                pdram_tiles[h] = None

            emit_gather(0)
            emit_gather(1)

            for h in range(H):
                ch, ro = divmod(h, 2)
                ro *= D
                rs = slice(ro, ro + D)
                a_sb = asb[h % 2]
                s_exp = sexp_tiles[h]
                if h + 2 < H:
                    emit_gather(h + 2)
                for ib in range(NIB):
                    isl = slice(ib * P, (ib + 1) * P)
                    jmax = 640 + ib * P
                    cp = cps.tile([P, FULL], FP32, tag="cp")
                    nc.tensor.matmul(
                        cp[:, 0:512], QuT[rs, ch, isl], KT[rs, ch, 0:512],
                        start=True, stop=True,
                    )
                    nc.tensor.matmul(
                        cp[:, 512:jmax], QuT[rs, ch, isl], KT[rs, ch, 512:jmax],
                        start=True, stop=True,
                    )
                    nc.scalar.activation(
                        a_sb[:, ib, :jmax], cp[:, :jmax], AF.Exp, scale=SCALE
                    )
                    z_t = work.tile([P, 1], FP32, tag="z_t")
                    # A = eC * eS with Z = sum_j A fused (InstTensorScalarPtr)
                    nc.vector.scalar_tensor_tensor(
                        out=a_sb[:, ib, :jmax],
                        in0=a_sb[:, ib, :jmax],
                        scalar=1.0,
                        in1=s_exp[:, ib, :jmax],
                        op0=ALU.mult,
                        op1=ALU.mult,
                        accum_out=z_t,
                    )
                    rz = work.tile([P, 1], FP32, tag="rz")
                    nc.vector.reciprocal(rz, z_t)
                    if ib == NIB - 1:
                        nc.scalar.mul(a_sb[:, ib, :jmax], a_sb[:, ib, :jmax], rz)
                    else:
                        nc.vector.tensor_scalar_mul(
                            a_sb[:, ib, :jmax], a_sb[:, ib, :jmax], rz
                        )
                sexp_tiles[h] = None
                # one full-head XBAR transpose: [128 i, (ib j)] -> [j%128, (ib jc), i%128]
                a_t = atp.tile([P, NIB, NJC, P], BF16, tag=f"at{h % 2}",
                               name=f"at{h % 2}")
                at_tiles[h] = a_t
                nc.sync.dma_start(a_t, a_sb, transpose=True)

                if h % 2 == 1:
                    hp = h // 2
                    av2 = [avp.tile([P, CUR], FP32, tag="av_a", name="av_a"),
                           avp.tile([P, CUR], FP32, tag="av_b", name="av_b")]
                    for jc in range(NJC):
                        ibmin = max(0, jc - 4)
                        for hh in range(2):
                            hx = 2 * hp + hh
                            nc.tensor.matmul(
                                av2[hh][hh * D : (hh + 1) * D, ibmin * P :],
                                V[:, jc, hx * D : (hx + 1) * D],
                                at_tiles[hx][:, ibmin:, jc, :],
                                start=(jc == 0),
                                stop=(jc == NJC - 1),
                                tile_position=(0, hh * D),
                            )
                    nc.vector.tensor_copy(AVT[0:D, hp, :], av2[0][0:D, :])
                    nc.vector.tensor_copy(AVT[D:P, hp, :], av2[1][D:P, :])

            # ---------------- Final projection ----------------
            with tc.tile_pool(name="fin", bufs=1) as fin:
                o_all = fin.tile([P, NIB, DIM], FP32, tag="o_all")
                for ib in range(NIB):
                    isl = slice(ib * P, (ib + 1) * P)
                    for eh in range(2):
                        esl = slice(eh * 512, (eh + 1) * 512)
                        fp = avp.tile([P, 512], FP32, tag=("av_a", "av_b")[eh],
                                      name="fp")
                        for fc in range(NCH):
                            nc.tensor.matmul(
                                fp, AVT[:, fc, isl], WPROJ[:, fc, esl],
                                start=(fc == 0), stop=(fc == NCH - 1),
                            )
                        nc.vector.tensor_tensor(
                            o_all[:, ib, esl], fp, bout_b[:, esl], ALU.add
                        )
                nc.sync.dma_start(out.rearrange("(ib p) e -> p ib e", p=P), o_all)

    if split_waits:
        _split_multiwait(nc)
    return nc


def _get_nc():
    global _BUILT
    if _BUILT is None:
        _BUILT = _build()
    return _BUILT


def _prep_host(inputs, pos_embedding, full_input, u, v, mask,
               W_kv, b_kv, W_q, b_q, W_pos, b_pos, W_proj, b_proj):
    f32 = np.float32
    W_k = np.ascontiguousarray(W_kv[:, : H * D])
    W_v = np.ascontiguousarray(W_kv[:, H * D :])
    b_k = b_kv[: H * D].astype(f32)
    b_v = b_kv[H * D :].astype(f32)
    bias_qu = (b_q + u.ravel()).astype(f32)
    bias_qv = (b_q + v.ravel()).astype(f32)
    b_out = (b_v @ W_proj + b_proj).astype(f32)

    bias_all = np.stack(
        [bias_qu.reshape(NCH, P), bias_qv.reshape(NCH, P),
         b_k.reshape(NCH, P), b_pos.astype(f32).reshape(NCH, P)], axis=0
    )  # [4, NCH, P]
    bias_all = np.ascontiguousarray(bias_all.transpose(2, 0, 1).reshape(P, 4 * NCH))
    wmats_np = np.concatenate([W_q, W_pos, W_k, W_v], axis=1).astype(nbf16)
    shared = {
        "wmats": wmats_np,
        "wproj": W_proj.astype(nbf16),
        "biases": bias_all.astype(f32),
        "bout": b_out.astype(nbf16),
    }
    pT_np = pos_embedding[:, 0].T
    in_maps = []
    for c in range(BS):
        m = dict(shared)
        m["acts"] = np.concatenate(
            [full_input[:, c].T, inputs[:, c].T, pT_np], axis=1
        ).astype(nbf16)
        in_maps.append(m)
    return in_maps


def kernel(**inputs):
    nc = _get_nc()
    in_maps = _prep_host(**{k: np.asarray(v) for k, v in inputs.items()})
    res = run_bass_kernel_spmd(nc, in_maps, list(range(BS)))
    out = np.stack([res.results[c]["out"] for c in range(BS)], axis=1)
    return np.ascontiguousarray(out.astype(np.float32))


if __name__ == "__main__":
    nc = _build()
    print("built ok")


# revision 15
# speedup vs baseline: 2.9811x; 1.0951x over previous
"""TransformerXL attention (AttentionXL) Bass kernel for Trainium2, 8 NeuronCores.

Sharding: pure data-parallel over batch (BS=8 -> 1 batch element per core).
All weights replicated per core; no collectives.

v6 design (fully fused pipeline; v5 was ~373us, v1 baseline ~389us):
  - exp factoring: A = exp((C+S)*s) = exp(C*s) * exp(S*s).  The position
    scores are exponentiated on the way out of PSUM, the rel-shift DMA
    gathers exp(S*s), and the combine is one DVE scalar_tensor_tensor
    (A = eC*eS with Z accumulated in the same instruction).  The causal
    mask becomes a multiplicative 0-fill on the diagonal 128-block.
  - Two fused phases with SBUF-lifetime-aware pools:
      phase 0: QT (kc-outer pairs) -> per-RT-chunk P-score production
               (matmul + expP + DRAM write).  RT/QvT/wq/wpos/xcT/pT die
               here, freeing SBUF+PSUM for phase 1.
      phase 1: per chunk ch: KT[ch+1] (one block ahead), C+softmax for
               heads 2ch/2ch+1, V slice, AV for the lagged head pair.
    Engine loads balance: scalar = expP (ph0) / expC (ph1), DVE =
    combine+norm, PE never waits on a same-block producer.
  - A^T via ONE DMA XBAR transpose per head ([128 i, (ib j)] ->
    [j%128, (ib jc), i%128]); AV reads 3-level [part, ib, u] APs.
    a_t rotates over 3 tiles so AV lags the transpose by a full head.
  - P matmuls/exps trimmed to the m-range the rel-shift reads
    (m >= 384-128*ib); persistent pa/a_sb tiles with one-time memsets
    keep every byte under the DMAs initialized (race-detector clean).
  - Per-matrix per-chunk input DMAs ordered wq,xc -> wpos,pT -> wk,wv,xT
    so the first QT matmul starts after ~3MB instead of 13.6MB.
  - Final projection bias via broadcast b_out tile + DVE add fused into
    the PSUM drain; per-i-block output DMAs.

Per-core algorithm (bf16 on the PE, fp32 PSUM accumulation):
  Host prep:  X^T, Xc^T, Pos^T, W_kv split into W_k/W_v, bias folds:
                bias_qu = b_q + u.ravel(), bias_qv = b_q + v.ravel()
                b_out   = b_v @ W_proj + b_proj  (softmax rows sum to 1)
  Device:
    KT = W_k^T @ X^T   [hd, j]   (+b_k)      RT = W_pos^T @ P^T [hd, m]
    QT = W_q^T @ Xc^T  [hd, i]  -> QuT/QvT   V  = X^T.T @ W_v   [j, hd]
    per head h:
      eP [i,m] = exp(QvT_h^T RT_h * s) -> DRAM
      eS [i,j] = ePflat[i*1023 + 511 + j]  (rel-shift gather), diag 0-mask
      eC [i,j] = exp(QuT_h^T KT_h * s)  (ScalarE from PSUM)
      A = eC*eS, Z = sum_j A  (DVE);  A *= 1/Z
      A^T via DMA XBAR transpose -> a_t [j%128, (ib,jc), i%128]
      O^T_h [d, i] = V_h^T A^T (PE, col-packed head pairs) -> AVT
    out[i,e] = AVT^T @ W_proj (+ b_out via DVE broadcast add), fp32.
"""

import os
import sys

for _p in (
    "/root/.axon_site",
    "/root/.axon_site/_ro/trn_rl_repo",
    "/root/.axon_site/_ro/pypackages",
    "/opt/trn_rl_repo",
):
    if os.path.isdir(_p) and _p not in sys.path:
        sys.path.append(_p)

import numpy as np
import ml_dtypes

import concourse.bass as bass
import concourse.mybir as mybir
import concourse.tile as tile
from concourse.bass_utils import run_bass_kernel_spmd

BF16 = mybir.dt.bfloat16
FP32 = mybir.dt.float32
AF = mybir.ActivationFunctionType
ALU = mybir.AluOpType
nbf16 = ml_dtypes.bfloat16

CUR, FULL, BS, DIM, H, D = 512, 1024, 8, 1024, 16, 64
PREV = FULL - CUR
SCALE = 1.0 / D**0.5
P = 128
NIB = CUR // P    # 4 query blocks
NJC = FULL // P   # 8 key chunks
NCH = DIM // P    # 8 dim chunks
NHP = H // 2      # 8 head pairs

_BUILT = None


def _mlo(ib):
    # lowest m the rel-shift gather reads within i-block ib
    return max(0, 384 - 128 * ib)


def _split_multiwait(nc):
    """walrus here encodes at most ONE sync wait per TPB instruction
    (NEURON_ISA_TPB_EVENTS has a single wait slot).  Split every
    multi-wait instruction: prepend same-engine NoOps carrying the
    extra waits, keep the last wait on the instruction itself."""
    n_split = 0
    for fn in nc.m.functions:
        for blk in fn.blocks:
            insts = list(blk.instructions)
            out = []
            for ins in insts:
                si = ins.sync_info
                if si is not None and si.on_wait and len(si.on_wait) > 1:
                    waits = list(si.on_wait)
                    for w in waits[:-1]:
                        nop = mybir.InstNoOp(
                            name=f"{ins.name}-ws{n_split}",
                            engine=ins.engine,
                            sync_info=mybir.SyncInfo(on_wait=[w], on_update=[]),
                            text_hint="waitsplit",
                        )
                        out.append(nop)
                        n_split += 1
                    ins.sync_info = mybir.SyncInfo(
                        on_wait=[waits[-1]],
                        on_update=list(si.on_update or []),
                    )
                out.append(ins)
            blk.instructions = out
    return n_split


def _build(split_waits=True):
    nc = bass.Bass()

    # acts: [X^T | Xc^T | Pos^T] cols; wmats: [W_q | W_pos | W_k | W_v] cols
    acts = nc.declare_dram_parameter("acts", [DIM, FULL + CUR + FULL], BF16, isOutput=False)
    wmats = nc.declare_dram_parameter("wmats", [DIM, 4 * DIM], BF16, isOutput=False)
    wproj = nc.declare_dram_parameter("wproj", [DIM, DIM], BF16, isOutput=False)
    # biases pre-laid-out on host: [p, 4*NCH] = qu | qv | k | pos chunks
    biases = nc.declare_dram_parameter("biases", [P, 4 * NCH], FP32, isOutput=False)
    bout = nc.declare_dram_parameter("bout", [DIM], BF16, isOutput=False)
    out = nc.declare_dram_parameter("out", [CUR, DIM], FP32, isOutput=True)

    with tile.TileContext(nc) as tc:
        from contextlib import ExitStack

        with ExitStack() as ctx:
            persist = ctx.enter_context(tc.tile_pool(name="persist", bufs=1))

            KT = persist.tile([P, NCH, FULL], BF16, tag="KT")
            V = persist.tile([P, NJC, DIM], BF16, tag="V")
            QuT = persist.tile([P, NCH, CUR], BF16, tag="QuT")
            AVT = persist.tile([P, NCH, CUR], BF16, tag="AVT")
            bout_b = persist.tile([P, DIM], BF16, tag="bout_b")
            bias_t = persist.tile([P, 4, NCH], FP32, tag="bias_t")  # qu|qv|k|pos

            mask_zero_reg = nc.gpsimd.to_reg(0.0)
            nc.sync.dma_start(bias_t, biases.rearrange("p (b c) -> p b c", b=4))
            nc.sync.dma_start(
                bout_b, bass.AP(tensor=bout, offset=0, ap=[[0, P], [1, DIM]])
            )

            pdram_tiles = [None] * H
            sexp_tiles = [None] * H
            dram = ctx.enter_context(tc.tile_pool(name="dram", bufs=16, space="DRAM"))
            # whole-kernel inputs: xT, wk, wv
            a2 = ctx.enter_context(tc.tile_pool(name="a2", bufs=1))
            xT_t = a2.tile([P, NCH, FULL], BF16, tag="xT")
            wk_t = a2.tile([P, NCH, DIM], BF16, tag="wk")
            wv_t = a2.tile([P, NCH, DIM], BF16, tag="wv")
            apsum = ctx.enter_context(tc.tile_pool(name="apsum", bufs=2, space="PSUM"))

            acts_r = acts.rearrange("(c p) f -> p c f", p=P)
            wmats_r = wmats.rearrange("(c p) f -> p c f", p=P)

            # ---------- phase 0: QT, RT + exp(P)-score production ----------
            with tc.tile_pool(name="a1", bufs=1) as a1, tc.tile_pool(
                name="pps", bufs=2, space="PSUM"
            ) as pps:
                RT = a1.tile([P, NCH, FULL], BF16, tag="RT")
                QvT = a1.tile([P, NCH, CUR], BF16, tag="QvT")
                pa = [a1.tile([P, NIB, FULL], BF16, tag="pa0", name="pa0"),
                      a1.tile([P, NIB, FULL], BF16, tag="pa1", name="pa1")]
                xcT_t = a1.tile([P, NCH, CUR], BF16, tag="xcT")
                pT_t = a1.tile([P, NCH, FULL], BF16, tag="pT")
                wq_t = a1.tile([P, NCH, DIM], BF16, tag="wq")
                wpos_t = a1.tile([P, NCH, DIM], BF16, tag="wpos")

                # input loads, consumer-ordered: (wq,xc) -> (wpos,pT) -> rest
                for c in range(NCH):
                    nc.sync.dma_start(wq_t[:, c : c + 1], wmats_r[:, c : c + 1, 0:DIM])
                    nc.sync.dma_start(
                        xcT_t[:, c : c + 1], acts_r[:, c : c + 1, FULL : FULL + CUR]
                    )
                for c in range(NCH):
                    nc.sync.dma_start(
                        wpos_t[:, c : c + 1], wmats_r[:, c : c + 1, DIM : 2 * DIM]
                    )
                    nc.sync.dma_start(
                        pT_t[:, c : c + 1], acts_r[:, c : c + 1, FULL + CUR :]
                    )
                for c in range(NCH):
                    nc.sync.dma_start(
                        wk_t[:, c : c + 1], wmats_r[:, c : c + 1, 2 * DIM : 3 * DIM]
                    )
                    nc.sync.dma_start(
                        wv_t[:, c : c + 1], wmats_r[:, c : c + 1, 3 * DIM : 4 * DIM]
                    )
                    nc.sync.dma_start(xT_t[:, c : c + 1], acts_r[:, c : c + 1, 0:FULL])

                for t in pa:
                    for ib in range(NIB - 1):
                        nc.vector.memset(t[:, ib, 0 : _mlo(ib)], 0.0)

                # Q^T [hd, i]: kc-outer pairs so matmuls consume chunks as
                # they land instead of waiting for the full load.
                for grp in range(4):
                    ocs = range(grp * 2, grp * 2 + 2)
                    pss = {
                        oc: apsum.tile([P, CUR], FP32, tag="aps", name=f"qps{oc}")
                        for oc in ocs
                    }
                    for kc in range(NCH):
                        for oc in ocs:
                            nc.tensor.matmul(
                                pss[oc],
                                wq_t[:, kc, oc * P : (oc + 1) * P],
                                xcT_t[:, kc, :],
                                start=(kc == 0),
                                stop=(kc == NCH - 1),
                            )
                    for oc in ocs:
                        nc.scalar.activation(
                            QuT[:, oc, :], pss[oc], AF.Identity,
                            bias=bias_t[:, 0, oc : oc + 1],
                        )
                        nc.scalar.activation(
                            QvT[:, oc, :], pss[oc], AF.Identity,
                            bias=bias_t[:, 1, oc : oc + 1],
                        )

                def emit_P(h):
                    """exp(P*s) production for head h: matmuls + expP + DRAM."""
                    ch, ro = divmod(h, 2)
                    ro *= D
                    rs = slice(ro, ro + D)
                    p_all = pa[h % 2]
                    for ib in range(NIB):
                        isl = slice(ib * P, (ib + 1) * P)
                        mlo = _mlo(ib)
                        pp = pps.tile([P, FULL], FP32, tag="pp", name="pp")
                        nc.tensor.matmul(
                            pp[:, mlo:512], QvT[rs, ch, isl], RT[rs, ch, mlo:512],
                            start=True, stop=True,
                        )
                        nc.tensor.matmul(
                            pp[:, 512:FULL], QvT[rs, ch, isl], RT[rs, ch, 512:FULL],
                            start=True, stop=True,
                        )
                        nc.scalar.activation(
                            p_all[:, ib, mlo:], pp[:, mlo:], AF.Exp, scale=SCALE
                        )
                    pdram = dram.tile([CUR, FULL], BF16, tag="pdram", name="pdram")
                    nc.sync.dma_start(
                        pdram.rearrange("(ib p) m -> p ib m", p=P), p_all
                    )
                    pdram_tiles[h] = pdram

                # R^T chunks, each followed by the P production it unblocks
                for ch in range(NCH):
                    for jh in range(2):
                        sl = slice(jh * 512, (jh + 1) * 512)
                        ps = apsum.tile([P, 512], FP32, tag="aps")
                        for kc in range(NCH):
                            nc.tensor.matmul(
                                ps,
                                wpos_t[:, kc, ch * P : (ch + 1) * P],
                                pT_t[:, kc, sl],
                                start=(kc == 0),
                                stop=(kc == NCH - 1),
                            )
                        nc.scalar.activation(
                            RT[:, ch, sl], ps, AF.Identity,
                            bias=bias_t[:, 3, ch : ch + 1],
                        )
                    emit_P(2 * ch)
                    emit_P(2 * ch + 1)

            # ---------- phase 1: KT + C/softmax + V + AV, fused ----------
            late = ctx.enter_context(tc.tile_pool(name="late", bufs=1))
            sall = ctx.enter_context(tc.tile_pool(name="sall", bufs=4))
            work = ctx.enter_context(tc.tile_pool(name="work", bufs=4))
            cps = ctx.enter_context(tc.tile_pool(name="cps", bufs=2, space="PSUM"))
            avp = ctx.enter_context(tc.tile_pool(name="avp", bufs=1, space="PSUM"))

            WPROJ = late.tile([P, NCH, DIM], BF16, tag="WPROJ")
            nc.sync.dma_start(WPROJ, wproj.rearrange("(c p) f -> p c f", p=P))
            # persistent exp(C)/A staging pair; beyond-jmax regions memset once
            asb = [late.tile([P, NIB, FULL], BF16, tag="as0", name="as0"),
                   late.tile([P, NIB, FULL], BF16, tag="as1", name="as1")]
            for t in asb:
                for ib in range(NIB - 1):
                    nc.vector.memset(t[:, ib, 640 + ib * P :], 0.0)
            # a_t rotation depth 4: AV(hp) runs a block after both its
            # heads' transposes, and no XBAR before it can touch their tiles
            atl = [late.tile([P, NIB, NJC, P], BF16, tag=f"at{k}", name=f"at{k}")
                   for k in range(4)]

            def emit_gather(h):
                """rel-shift gather of exp(S*s) + multiplicative diag mask."""
                pdram = pdram_tiles[h]
                s_exp = sall.tile([P, NIB, FULL], BF16, tag="s_exp", name="s_exp")
                sh_ap = bass.AP(
                    tensor=pdram.tensor,
                    offset=pdram.offset + (PREV - 1),
                    ap=[[FULL - 1, P], [(FULL - 1) * P, NIB], [1, FULL]],
                )
                nc.sync.dma_start(s_exp, sh_ap)
                # causal mask: zero the over-diagonal in the diagonal block
                # (keep iff u - j' >= 0; u = i%128, j' = j-512-128*ib)
                for ib in range(NIB):
                    j0 = 512 + ib * P
                    nc.gpsimd.affine_select(
                        out=s_exp[:, ib, j0 : j0 + P],
                        in_=s_exp[:, ib, j0 : j0 + P],
                        compare_op=ALU.is_ge,
                        fill=mask_zero_reg,
                        base=0,
                        channel_multiplier=1,
                        pattern=[[-1, P]],
                    )
                sexp_tiles[h] = s_exp
                pdram_tiles[h] = None

            def emit_KT(ch):
                for jh in range(2):
                    sl = slice(jh * 512, (jh + 1) * 512)
                    ps = apsum.tile([P, 512], FP32, tag="aps")
                    for kc in range(NCH):
                        nc.tensor.matmul(
                            ps,
                            wk_t[:, kc, ch * P : (ch + 1) * P],
                            xT_t[:, kc, sl],
                            start=(kc == 0),
                            stop=(kc == NCH - 1),
                        )
                    nc.scalar.activation(
                        KT[:, ch, sl], ps, AF.Identity,
                        bias=bias_t[:, 2, ch : ch + 1],
                    )

            def emit_softmax(h):
                ch, ro = divmod(h, 2)
                ro *= D
                rs = slice(ro, ro + D)
                a_sb = asb[h % 2]
                s_exp = sexp_tiles[h]
                for ib in range(NIB):
                    isl = slice(ib * P, (ib + 1) * P)
                    jmax = 640 + ib * P
                    cp = cps.tile([P, FULL], FP32, tag="cp")
                    nc.tensor.matmul(
                        cp[:, 0:512], QuT[rs, ch, isl], KT[rs, ch, 0:512],
                        start=True, stop=True,
                    )
                    nc.tensor.matmul(
                        cp[:, 512:jmax], QuT[rs, ch, isl], KT[rs, ch, 512:jmax],
                        start=True, stop=True,
                    )
                    nc.scalar.activation(
                        a_sb[:, ib, :jmax], cp[:, :jmax], AF.Exp, scale=SCALE
                    )
                    z_t = work.tile([P, 1], FP32, tag="z_t")
                    # A = eC * eS with Z = sum_j A fused
                    nc.vector.scalar_tensor_tensor(
                        out=a_sb[:, ib, :jmax],
                        in0=a_sb[:, ib, :jmax],
                        scalar=1.0,
                        in1=s_exp[:, ib, :jmax],
                        op0=ALU.mult,
                        op1=ALU.mult,
                        accum_out=z_t,
                    )
                    rz = work.tile([P, 1], FP32, tag="rz")
                    nc.vector.reciprocal(rz, z_t)
                    nc.vector.tensor_scalar_mul(
                        a_sb[:, ib, :jmax], a_sb[:, ib, :jmax], rz
                    )
                sexp_tiles[h] = None
                # one full-head XBAR: [128 i, (ib j)] -> [j%128, (ib jc), i%128]
                a_t = atl[h % 4]
                nc.sync.dma_start(a_t, a_sb, transpose=True)
                return a_t

            at_of = [None] * H

            def emit_AV(hp):
                av2 = [avp.tile([P, CUR], FP32, tag="av_a", name="av_a"),
                       avp.tile([P, CUR], FP32, tag="av_b", name="av_b")]
                for jc in range(NJC):
                    ibmin = max(0, jc - 4)
                    for hh in range(2):
                        hx = 2 * hp + hh
                        nc.tensor.matmul(
                            av2[hh][hh * D : (hh + 1) * D, ibmin * P :],
                            V[:, jc, hx * D : (hx + 1) * D],
                            at_of[hx][:, ibmin:, jc, :],
                            start=(jc == 0),
                            stop=(jc == NJC - 1),
                            tile_position=(0, hh * D),
                        )
                nc.vector.tensor_copy(AVT[0:D, hp, :], av2[0][0:D, :])
                nc.vector.tensor_copy(AVT[D:P, hp, :], av2[1][D:P, :])

            emit_KT(0)
            emit_gather(0)
            emit_gather(1)
            for ch in range(NCH):
                if ch + 1 < NCH:
                    emit_KT(ch + 1)
                if 2 * ch + 2 < H:
                    emit_gather(2 * ch + 2)
                at_of[2 * ch] = emit_softmax(2 * ch)
                if 2 * ch + 3 < H:
                    emit_gather(2 * ch + 3)
                at_of[2 * ch + 1] = emit_softmax(2 * ch + 1)
                # V slices: AV(hp) needs its full head-half over all jc, so
                # the low half lands in blocks 0-1 (before AV(0) at block 1)
                # and the high half spreads over blocks 2-5 (before AV(4)).
                vs = {0: [(0, j) for j in range(4)],
                      1: [(0, j) for j in range(4, 8)],
                      2: [(1, 0), (1, 1)], 3: [(1, 2), (1, 3)],
                      4: [(1, 4), (1, 5)], 5: [(1, 6), (1, 7)]}.get(ch, [])
                for mh, jc in vs:
                    sl = slice(mh * 512, (mh + 1) * 512)
                    ps = apsum.tile([P, 512], FP32, tag="aps")
                    for kc in range(NCH):
                        nc.tensor.matmul(
                            ps,
                            xT_t[:, kc, jc * P : (jc + 1) * P],
                            wv_t[:, kc, sl],
                            start=(kc == 0),
                            stop=(kc == NCH - 1),
                        )
                    nc.vector.tensor_copy(V[:, jc, sl], ps)
                if ch >= 1:
                    emit_AV(ch - 1)
            emit_AV(NHP - 1)

            # ---------------- Final projection ----------------
            with tc.tile_pool(name="fin", bufs=2) as fin:
                out_r = out.rearrange("(ib p) e -> p ib e", p=P)
                for ib in range(NIB):
                    isl = slice(ib * P, (ib + 1) * P)
                    o_ib = fin.tile([P, DIM], FP32, tag="o_ib", name="o_ib")
                    for eh in range(2):
                        esl = slice(eh * 512, (eh + 1) * 512)
                        fp = avp.tile([P, 512], FP32, tag=("av_a", "av_b")[eh],
                                      name="fp")
                        for fc in range(NCH):
                            nc.tensor.matmul(
                                fp, AVT[:, fc, isl], WPROJ[:, fc, esl],
                                start=(fc == 0), stop=(fc == NCH - 1),
                            )
                        nc.vector.tensor_tensor(
                            o_ib[:, esl], fp, bout_b[:, esl], ALU.add
                        )
                    nc.sync.dma_start(out_r[:, ib, :], o_ib)

    if split_waits:
        _split_multiwait(nc)
    return nc


def _get_nc():
    global _BUILT
    if _BUILT is None:
        _BUILT = _build()
    return _BUILT


def _prep_host(inputs, pos_embedding, full_input, u, v, mask,
               W_kv, b_kv, W_q, b_q, W_pos, b_pos, W_proj, b_proj):
    f32 = np.float32
    W_k = np.ascontiguousarray(W_kv[:, : H * D])
    W_v = np.ascontiguousarray(W_kv[:, H * D :])
    b_k = b_kv[: H * D].astype(f32)
    b_v = b_kv[H * D :].astype(f32)
    bias_qu = (b_q + u.ravel()).astype(f32)
    bias_qv = (b_q + v.ravel()).astype(f32)
    b_out = (b_v @ W_proj + b_proj).astype(f32)

    bias_all = np.stack(
        [bias_qu.reshape(NCH, P), bias_qv.reshape(NCH, P),
         b_k.reshape(NCH, P), b_pos.astype(f32).reshape(NCH, P)], axis=0
    )  # [4, NCH, P]
    bias_all = np.ascontiguousarray(bias_all.transpose(2, 0, 1).reshape(P, 4 * NCH))
    wmats_np = np.concatenate([W_q, W_pos, W_k, W_v], axis=1).astype(nbf16)
    shared = {
        "wmats": wmats_np,
        "wproj": W_proj.astype(nbf16),
        "biases": bias_all.astype(f32),
        "bout": b_out.astype(nbf16),
    }
    pT_np = pos_embedding[:, 0].T
    in_maps = []
    for c in range(BS):
        m = dict(shared)
        m["acts"] = np.concatenate(
            [full_input[:, c].T, inputs[:, c].T, pT_np], axis=1
        ).astype(nbf16)
        in_maps.append(m)
    return in_maps


def kernel(**inputs):
    nc = _get_nc()
    in_maps = _prep_host(**{k: np.asarray(v) for k, v in inputs.items()})
    res = run_bass_kernel_spmd(nc, in_maps, list(range(BS)))
    out = np.stack([res.results[c]["out"] for c in range(BS)], axis=1)
    return np.ascontiguousarray(out.astype(np.float32))


if __name__ == "__main__":
    nc = _build()
    print("built ok")
